# revision 59
# baseline (speedup 1.0000x reference)
"""Trainium2 Bass kernel for GQA attention (B=1, S=2048, D=2048, 32 Q heads,
8 KV heads, head_dim 64), 8-way tensor parallel over heads.

v2: fully software-pipelined single pass.
  - Core c owns Q heads 4c..4c+3 and KV head c (GQA maps exactly).
  - Emission interleaves projection (per 512-col n-group), attention
    (per q-group g=n, heads pair-interleaved), and the wo epilogue so the
    Scalar engine (softmax exp, the critical resource) runs dense from ~10us.
  - Scores S^T[k',q] = K^T Q are K=64 matmuls: two per PSUM pair run
    CONCURRENTLY in disjoint PE row-groups via tile_position (0,0)/(64,0),
    with K and Q duplicated on partitions 64..127.
  - exp is trimmed by the causal lead offset of the first tile of each pair
    (tiles sorted by lead desc); masked lead columns are zeroed by the
    multiplicative mask tiles (pt tiles memset once to avoid stale NaNs).
  - Softmax denominator rides as a 65th row of [V|1]^T P^T; normalization is
    reciprocal (DVE, direct from PSUM) -> bf16 -> ones[1,64]^T @ rec PE
    broadcast -> DVE multiply. GpSimd runs mask multiplies + PSUM->SBUF
    copies; its FIFO ends with the two AllToAll collectives only.
  - Two AllToAlls (heads 0-1, then 2-3); wo accumulates all 16 contraction
    tiles in PSUM across the A2A#2 wait (no SBUF accumulator).
  - xt is loaded as 64 [128,512] chunks (n-major) and its SBUF slots are
    reused for wo column chunks once the projection consumed them.
"""

import os
import sys

import numpy as np

for _p in ("/opt/trn_rl_repo", "/root/.axon_site/_ro/trn_rl_repo"):
    if os.path.isdir(_p) and _p not in sys.path:
        sys.path.insert(0, _p)

import ml_dtypes  # noqa: E402

from concourse import bacc, mybir, tile  # noqa: E402
from concourse.bass_utils import run_bass_kernel_spmd  # noqa: E402

BF16 = mybir.dt.bfloat16
F32 = mybir.dt.float32

S = 2048          # sequence length
D = 2048          # model dim
HD = 64           # head dim
NH = 32           # query heads
NKV = 8           # kv heads
NC = 8            # cores
HL = NH // NC     # q heads per core = 4
P = 128
QG = 512          # q-group width (score-tile free dim)
NG = S // QG      # 4 q groups
NT = S // P       # 16 k'-tiles
KD = D // P       # 16 contraction tiles for D-reductions
SR = S // NC      # 256 output rows per core
NE = D // QG      # 4 output column chunks

_bf = ml_dtypes.bfloat16


def _classify_mask(mask):
    """Per-tile slot plan. A slot computes scores for one k'-tile for TWO
    heads at once (partition halves, concurrent PE row groups). Per q-group
    g: non-skip tiles sorted by causal lead desc. Per tile: exp offset,
    duplicated [m_t|m_t] multiplicative mask index (None if fully passing)
    and multiply width."""
    mexp = np.exp(np.minimum(mask.astype(np.float64), 50.0)).astype(np.float32).T
    uniq = []
    uniq_keys = {}
    slots = {}
    for g in range(NG):
        sl = []
        for t in range(NT):
            tl = mexp[P * t:P * (t + 1), QG * g:QG * (g + 1)]
            if np.all(tl == 0.0):
                continue
            if np.all(tl == 1.0):
                sl.append((t, 0, None, 0))
                continue
            live = np.where((tl != 0.0).any(axis=0))[0]
            lead = (int(live[0]) // 8) * 8
            ne = np.where((tl != 1.0).any(axis=0))[0]
            w = min(QG, ((int(ne[-1]) + 1) + 3) // 4 * 4)
            comb = np.concatenate([tl, tl], axis=1).astype(_bf)
            key = comb.tobytes()
            if key not in uniq_keys:
                uniq_keys[key] = len(uniq)
                uniq.append(comb)
            sl.append((t, lead, uniq_keys[key], w))
        slots[g] = sorted(sl, key=lambda s: (-s[1], s[0]))
    return slots, uniq


def _build_nc(slots, n_uniq, dbg=False):
    nc = bacc.Bacc("TRN2", target_bir_lowering=False, debug=False,
                   num_devices=NC)

    xt_d = nc.dram_tensor("xt", [D, S], BF16, kind="ExternalInput")
    wqkv_d = nc.dram_tensor("wqkv", [D, HL * HD + 2 * HD], BF16,
                            kind="ExternalInput")
    wo_d = nc.dram_tensor("wo", [D, D], BF16, kind="ExternalInput")
    cos_d = nc.dram_tensor("cos2", [P, S], BF16, kind="ExternalInput")
    sin_d = nc.dram_tensor("sin2", [P, S], BF16, kind="ExternalInput")
    ident_d = nc.dram_tensor("ident", [P, P], BF16, kind="ExternalInput")
    pswap_d = nc.dram_tensor("pswap", [P, P], BF16, kind="ExternalInput")
    pkd_d = nc.dram_tensor("pkd", [P, P], BF16, kind="ExternalInput")
    pks_d = nc.dram_tensor("pks", [P, P], BF16, kind="ExternalInput")
    mt_d = None
    if n_uniq:
        mt_d = nc.dram_tensor("mtiles", [n_uniq, P, 2 * QG], BF16,
                              kind="ExternalInput")
    out_d = nc.dram_tensor("out", [SR, D], F32, kind="ExternalOutput")
    dbg_d = {}
    if dbg:
        dbg_d["kt"] = nc.dram_tensor("dbg_kt", [NG, P, QG], BF16,
                                     kind="ExternalOutput")
        dbg_d["qt"] = nc.dram_tensor("dbg_qt", [2, NG, P, 2, QG], BF16,
                                     kind="ExternalOutput")
        dbg_d["v"] = nc.dram_tensor("dbg_v", [NT, P, HD + 1], BF16,
                                    kind="ExternalOutput")
        dbg_d["asb"] = nc.dram_tensor("dbg_asb", [HL, NG, HD, QG], BF16,
                                      kind="ExternalOutput")
        dbg_d["att"] = nc.dram_tensor("dbg_att", [HL, NG, HD + 1, QG], F32,
                                      kind="ExternalOutput")

    with tile.TileContext(nc) as tc:
        with (
            tc.tile_pool(name="xtp", bufs=1) as xtp,      # xt chunks then wo chunks
            tc.tile_pool(name="const", bufs=1) as const,
            tc.tile_pool(name="work", bufs=3) as work,
            tc.tile_pool(name="pt", bufs=4) as ptpool,
            tc.tile_pool(name="ps_sc", bufs=2, space="PSUM") as ps_sc,   # scores: [128,1024] f32 = 2 banks ea
            tc.tile_pool(name="ps_pj", bufs=2, space="PSUM") as ps_pj,   # proj/sw/pv/rec64: 1 bank ea
            tc.tile_pool(name="ps_att", bufs=2, space="PSUM") as ps_att,  # att accum: 1 bank ea
            tc.tile_pool(name="dram", bufs=1, space="DRAM") as dram,
        ):
            # ---- bulk loads. The Sync sequencer pays ~600ns per DMA trigger,
            # so triggers are few+fat; wq/wkv ride the Activation HWDGE queue
            # (idle until the first exp). xt n=0 is split thin so the first
            # projection group lands in ~8us; the rest is one fat DMA per k.
            wqkv_sb = [const.tile([P, HL * HD + 2 * HD], BF16, tag=f"wq{k}",
                                  name=f"wq{k}") for k in range(KD)]
            xt0 = [xtp.tile([P, 2 * QG], BF16, tag=f"x0{k}", name=f"xt0_{k}")
                   for k in range(KD)]
            for k in range(KD):
                nc.sync.dma_start(wqkv_sb[k][:],
                                  wqkv_d.ap()[P * k:P * (k + 1), :])
                nc.sync.dma_start(xt0[k][:],
                                  xt_d.ap()[P * k:P * (k + 1), 0:2 * QG])
            cos2 = const.tile([P, S], BF16)
            sin2 = const.tile([P, S], BF16)
            ident = const.tile([P, P], BF16)
            pswap = const.tile([P, P], BF16)
            pkd = const.tile([P, P], BF16)
            pks = const.tile([P, P], BF16)
            nc.sync.dma_start(cos2[:], cos_d.ap())
            nc.sync.dma_start(sin2[:], sin_d.ap())
            nc.sync.dma_start(ident[:], ident_d.ap())
            nc.sync.dma_start(pswap[:], pswap_d.ap())
            nc.sync.dma_start(pkd[:], pkd_d.ap())
            nc.sync.dma_start(pks[:], pks_d.ap())
            xtR = [xtp.tile([P, 2 * QG], BF16, tag=f"b{k}", name=f"xtR{k}")
                   for k in range(KD)]
            for k in range(KD):
                nc.sync.dma_start(xtR[k][:],
                                  xt_d.ap()[P * k:P * (k + 1), 2 * QG:S])
            mtt = [const.tile([P, 2, QG], BF16, tag=f"mt{u}", name=f"mt{u}")
                   for u in range(n_uniq)]
            for u in range(n_uniq):
                nc.sync.dma_start(mtt[u][:], mt_d.ap()[u])
            wo_sb = [None] * KD

            def xt_slice(k, n):
                if n < 2:
                    return xt0[k][:, QG * n:QG * (n + 1)]
                return xtR[k][:, QG * (n - 2):QG * (n - 1)]

            # ---- small consts / memsets (gpsimd is idle at start) ----
            ones = const.tile([1, HD], BF16)
            nc.gpsimd.memset(ones[:], 1.0)
            pts_tiles = [ptpool.tile([P, 2, QG], BF16, tag="pt",
                                     name=f"pt{i}") for i in range(4)]
            for t_ in pts_tiles:
                nc.gpsimd.memset(t_[:], 0.0)
            v_sb = [const.tile([P, HD + 1], BF16, tag=f"v{t}", name=f"v{t}")
                    for t in range(NT)]
            for t in range(NT):
                nc.gpsimd.memset(v_sb[t][:, HD:HD + 1], 1.0)

            # persistent projection outputs (fine-grained tiles for dep tracking)
            kt = [const.tile([P, QG], BF16, tag=f"kt{n}", name=f"kt{n}")
                  for n in range(NG)]
            qt = [[const.tile([P, QG], BF16, tag=f"qt{m}_{n}",
                              name=f"qt{m}_{n}") for n in range(NG)]
                  for m in range(2)]

            a2a_in = [dram.tile([NC, 2 * HD, SR], BF16, tag=f"a2ai{i}",
                                name=f"a2ai{i}") for i in range(2)]
            a2a_out = [dram.tile([NC, 2 * HD, SR], BF16, tag=f"a2ao{i}",
                                 name=f"a2ao{i}") for i in range(2)]

            pt_i = [0]

            def proj(m, n):
                """m=0: q heads 0-1; m=1: q heads 2-3; m=2: K|V.
                RoPE: rot = raw*cos + perm(raw)*sin, the permutation done on
                the PE. For K|V the permutations also duplicate K onto both
                partition halves, so kt needs no SBUF-shift DMA."""
                nsl = slice(QG * n, QG * (n + 1))
                ps = ps_pj.tile([P, QG], F32, tag="pj", name=f"pj{m}_{n}")
                for k in range(KD):
                    lhsT = (wqkv_sb[k][:, P * m:P * (m + 1)] if m < 2
                            else wqkv_sb[k][:, HL * HD:HL * HD + 2 * HD])
                    nc.tensor.matmul(ps[:, 0:QG], lhsT, xt_slice(k, n),
                                     start=(k == 0), stop=(k == KD - 1))
                raw = work.tile([P, QG], BF16, tag="raw")
                nc.vector.tensor_copy(raw[:], ps[:, 0:QG])
                if m < 2:
                    sw = ps_pj.tile([P, QG], F32, tag="pj", name=f"sw{m}_{n}")
                    nc.tensor.matmul(sw[:], pswap[:], raw[:], start=True,
                                     stop=True)
                    t1 = work.tile([P, QG], BF16, tag="t1")
                    nc.gpsimd.tensor_mul(t1[:], raw[:], cos2[:, nsl])
                    t2 = work.tile([P, QG], BF16, tag="t2")
                    nc.vector.tensor_mul(t2[:], sw[:], sin2[:, nsl])
                    nc.vector.tensor_add(qt[m][n][:], t1[:], t2[:])
                else:
                    kd = ps_pj.tile([P, QG], F32, tag="pj", name=f"kd{n}")
                    nc.tensor.matmul(kd[:], pkd[:], raw[:], start=True,
                                     stop=True)
                    ks = ps_pj.tile([P, QG], F32, tag="pj", name=f"ks{n}")
                    nc.tensor.matmul(ks[:], pks[:], raw[:], start=True,
                                     stop=True)
                    t1 = work.tile([P, QG], BF16, tag="t1")
                    nc.vector.tensor_mul(t1[:], kd[:], cos2[:, nsl])
                    t2 = work.tile([P, QG], BF16, tag="t2")
                    nc.vector.tensor_mul(t2[:], ks[:], sin2[:, nsl])
                    nc.vector.tensor_add(kt[n][:], t1[:], t2[:])
                    for j in range(4):
                        t = 4 * n + j
                        pv = ps_pj.tile([P, HD], BF16, tag="pj",
                                        name=f"pv{t}")
                        nc.tensor.transpose(pv[:],
                                            raw[HD:P, P * j:P * (j + 1)],
                                            ident[HD:P, 0:HD])
                        nc.vector.tensor_copy(v_sb[t][:, 0:HD], pv[:])

            def attn_slot(mq, g, si, attA, attB):
                t, off, u, w = slots[g][si]
                last = si == len(slots[g]) - 1
                tsl = slice(P * (t % 4), P * (t % 4) + P)
                psc = ps_sc.tile([P, 2, QG], F32, tag="sc",
                                 name=f"s{mq}_{g}_{si}")
                for i in range(2):
                    nc.tensor.matmul(
                        psc[:, i, :],
                        kt[t // 4][HD * i:HD * (i + 1), tsl],
                        qt[mq][g][HD * i:HD * (i + 1), :],
                        start=True, stop=True,
                        tile_position=(HD * i, 0))
                pts = pts_tiles[pt_i[0] % 4]
                pt_i[0] += 1
                nc.scalar.activation(pts[:, :, off:QG], psc[:, :, off:QG],
                                     mybir.ActivationFunctionType.Exp,
                                     scale=0.125)
                if u is not None:
                    nc.vector.tensor_mul(pts[:, :, 0:w], pts[:, :, 0:w],
                                         mtt[u][:, :, 0:w])
                nc.tensor.matmul(attA[:], v_sb[t][:], pts[:, 0, :],
                                 start=(si == 0), stop=last)
                nc.tensor.matmul(attB[:], v_sb[t][:], pts[:, 1, :],
                                 start=(si == 0), stop=last)

            def attn_fin(h, g, att):
                if dbg:
                    attf = work.tile([HD + 1, QG], F32, tag="attf")
                    nc.vector.tensor_copy(attf[:], att[:])
                    nc.sync.dma_start(dbg_d["att"].ap()[h, g], attf[:])
                # den row sits at att partition HD; the custom-DVE reciprocal
                # drops nonzero base partitions, so DMA it to partition 0.
                den = work.tile([1, QG], F32, tag="den")
                nc.scalar.copy(den[:], att[HD:HD + 1, :])
                rec = work.tile([1, QG], F32, tag="rec")
                nc.vector.reciprocal_approx_fast(rec[:], den[:])
                recb = work.tile([1, QG], BF16, tag="recb")
                nc.vector.tensor_copy(recb[:], rec[:])
                rec64 = ps_pj.tile([HD, QG], F32, tag="pj",
                                   name=f"rb{h}_{g}")
                nc.tensor.matmul(rec64[:], ones[:], recb[:],
                                 start=True, stop=True)
                rec64s = work.tile([HD, QG], BF16, tag="rec64s")
                nc.vector.tensor_copy(rec64s[:], rec64[:])
                asb = work.tile([HD, QG], BF16, tag="asb")
                nc.vector.tensor_mul(asb[:], att[0:HD, :], rec64s[:])
                if dbg:
                    nc.sync.dma_start(dbg_d["asb"].ap()[h, g], asb[:])
                buf = a2a_in[h // 2]
                hr = HD * (h % 2)
                nc.sync.dma_start(buf[2 * g, hr:hr + HD, :], asb[:, 0:SR])
                nc.sync.dma_start(buf[2 * g + 1, hr:hr + HD, :],
                                  asb[:, SR:2 * SR])

            def attn_hpair(ha, hb, g, mask_eng):
                """Head-pair chains for (ha, hb) = (2mq, 2mq+1) on group g;
                each slot computes both heads concurrently. Returns a
                finisher closure to emit later (FIFO slack)."""
                mq = ha // 2
                attA = ps_att.tile([HD + 1, QG], F32, tag="att",
                                   name=f"att{ha}_{g}")
                attB = ps_att.tile([HD + 1, QG], F32, tag="att",
                                   name=f"att{hb}_{g}")
                for si in range(len(slots[g])):
                    attn_slot(mq, g, si, attA, attB)

                def fin():
                    attn_fin(ha, g, attA)
                    attn_fin(hb, g, attB)
                return fin



            # ================= main pipeline =================
            # Phase 1: heads 0-1 across all groups -> A2A#1 fires early.
            # Phase 2: q23 projection + heads 2-3; wo epilogue overlaps A2A#2.
            # Finisher ordering rule: a chain's finisher (whose PE broadcast
            # frees its att PSUM slot) must be emitted BEFORE any later chain
            # that waits on that slot, else the PE FIFO deadlocks.
            fin_prev = None
            for n in range(NG):
                proj(2, n)                       # K|V for group n
                proj(0, n)                       # q heads 0-1
                if fin_prev is not None:
                    fin_prev()                   # (h0,h1, n-1)
                fin_prev = attn_hpair(0, 1, n, nc.gpsimd)
            proj(1, 0)                           # prefetch q23 group 0
            fin_prev()                           # (h0,h1, g=3)
            # wo even k-tiles (deferred so they don't steal xtR bandwidth)
            for i in range(NC):
                t_ = const.tile([P, D], BF16, tag=f"we{i}", name=f"wo{2 * i}")
                wo_sb[2 * i] = t_
                nc.sync.dma_start(t_[:], wo_d.ap()[P * 2 * i:P * (2 * i + 1), :])
            nc.gpsimd.collective_compute(
                "AllToAll", mybir.AluOpType.bypass,
                replica_groups=[list(range(NC))],
                ins=[a2a_in[0].opt()], outs=[a2a_out[0].opt()])
            # ao tiles reuse the (dead after proj n=0) xt0 slots
            ao_sb = [[xtp.tile([P, SR], BF16, tag=f"x0{2 * i + hp}",
                               name=f"ao{hp}_{i}") for i in range(NC)]
                     for hp in range(2)]
            fin_prev = None
            for n in range(NG):
                if n < NG - 1:
                    proj(1, n + 1)               # q heads 2-3, next group
                if n == NG - 1:
                    # all xtR readers emitted: wo odd k-tiles into the slots,
                    # then the ao0 loads (gated on A2A#1 completion)
                    for i in range(NC):
                        t_ = xtp.tile([P, D], BF16, tag=f"b{2 * i + 1}",
                                      name=f"wo{2 * i + 1}")
                        wo_sb[2 * i + 1] = t_
                        nc.sync.dma_start(
                            t_[:], wo_d.ap()[P * (2 * i + 1):
                                             P * (2 * i + 2), :])
                if fin_prev is not None:
                    fin_prev()                   # (h2,h3, n-1)
                fin_prev = attn_hpair(2, 3, n, nc.vector)
            fin_prev()                           # (h2,h3, g=3)
            for i in range(NC):
                nc.sync.dma_start(ao_sb[0][i][:], a2a_out[0][i])
            if dbg:
                for n in range(NG):
                    nc.sync.dma_start(dbg_d["kt"].ap()[n], kt[n][:])
                    for m in range(2):
                        nc.sync.dma_start(dbg_d["qt"].ap()[m, n], qt[m][n][:])
                for t in range(NT):
                    nc.sync.dma_start(dbg_d["v"].ap()[t], v_sb[t][:])

            # second collective
            nc.gpsimd.collective_compute(
                "AllToAll", mybir.AluOpType.bypass,
                replica_groups=[list(range(NC))],
                ins=[a2a_in[1].opt()], outs=[a2a_out[1].opt()])
            for i in range(NC):
                nc.sync.dma_start(ao_sb[1][i][:], a2a_out[1][i])

            # ---- wo epilogue: 8 PSUM accumulators over all 16 k-tiles ----
            po = []
            for j in range(2):
                t_ = ps_sc.tile([P, 2 * QG], F32, tag="sc", name=f"po_sc{j}")
                po += [t_[:, 0:QG], t_[:, QG:2 * QG]]
            for j in range(2):
                t_ = ps_att.tile([P, QG], F32, tag="att", name=f"po_at{j}")
                po.append(t_[:])
            for j in range(2):
                t_ = ps_pj.tile([P, QG], F32, tag="pj", name=f"po_pj{j}")
                po.append(t_[:])
            # po index = sm*NE + ec. Half 0 overlaps the A2A#2 wait; half 1
            # finishes each accumulator in turn so the stores pipeline out.
            for i in range(NC):
                for sm in range(SR // P):
                    for ec in range(NE):
                        nc.tensor.matmul(
                            po[sm * NE + ec],
                            ao_sb[0][i][:, P * sm:P * (sm + 1)],
                            wo_sb[2 * i][:, QG * ec:QG * (ec + 1)],
                            start=(i == 0), stop=False,
                            skip_group_check=True)
            for sm in range(SR // P):
                for ec in range(NE):
                    for i in range(NC):
                        nc.tensor.matmul(
                            po[sm * NE + ec],
                            ao_sb[1][i][:, P * sm:P * (sm + 1)],
                            wo_sb[2 * i + 1][:, QG * ec:QG * (ec + 1)],
                            start=False, stop=(i == NC - 1),
                            skip_group_check=True)
                    osb = work.tile([P, QG], F32, tag="osb")
                    nc.vector.tensor_copy(osb[:], po[sm * NE + ec])
                    nc.sync.dma_start(
                        out_d.ap()[P * sm:P * (sm + 1),
                                   QG * ec:QG * (ec + 1)], osb[:])

    nc.compile()
    return nc


_CACHE = {}


def _get_compiled(mask):
    slots, uniq = _classify_mask(mask)
    key = tuple(sorted((g, tuple(sl)) for g, sl in slots.items()))
    if key not in _CACHE:
        _CACHE[key] = _build_nc(slots, len(uniq))
    return _CACHE[key], uniq


def _host_prep(x, freqs_cos, freqs_sin, mask, wq, wk, wv, wo, uniq):
    xt = np.ascontiguousarray(x[0].T).astype(_bf)
    perm = np.concatenate([np.arange(0, HD, 2), np.arange(1, HD, 2)])
    cosT = np.ascontiguousarray(freqs_cos.T)            # [32, S]
    sinT = np.ascontiguousarray(freqs_sin.T)
    cos2 = np.tile(cosT, (4, 1)).astype(_bf)            # [128, S]
    sin2 = np.tile(np.concatenate([-sinT, sinT], axis=0), (2, 1)).astype(_bf)
    ident = np.zeros((P, P), dtype=_bf)
    ident[0:HD, 0:HD] = np.eye(HD, dtype=_bf)
    ident[HD:P, 0:HD] = np.eye(HD, dtype=_bf)
    pswap = np.zeros((P, P), dtype=_bf)
    pkd = np.zeros((P, P), dtype=_bf)
    pks = np.zeros((P, P), dtype=_bf)
    for i in range(P):
        b, r = i // 32, i % 32
        pswap[32 * (b ^ 1) + r, i] = 1
        h64 = i % HD
        pkd[h64, i] = 1
        b2, r2 = h64 // 32, h64 % 32
        pks[32 * (b2 ^ 1) + r2, i] = 1
    wo_b = np.ascontiguousarray(wo).astype(_bf)
    mt = (np.stack(uniq, axis=0) if uniq
          else np.zeros((0, P, 2 * QG), dtype=_bf))

    in_maps = []
    for c in range(NC):
        qcols = np.concatenate(
            [HD * (HL * c + h) + perm for h in range(HL)])
        wqkv_c = np.ascontiguousarray(np.concatenate(
            [wq[:, qcols], wk[:, HD * c + perm],
             wv[:, HD * c:HD * (c + 1)]], axis=1)).astype(_bf)
        m = {"xt": xt, "wqkv": wqkv_c,
             "wo": wo_b, "cos2": cos2, "sin2": sin2, "ident": ident,
             "pswap": pswap, "pkd": pkd, "pks": pks}
        if len(uniq):
            m["mtiles"] = mt
        in_maps.append(m)
    return in_maps


def run(x, freqs_cos, freqs_sin, mask, wq, wk, wv, wo, trace=False):
    x = np.asarray(x, dtype=np.float32)
    mask = np.asarray(mask, dtype=np.float32)
    nc, uniq = _get_compiled(mask)
    in_maps = _host_prep(np.asarray(x), np.asarray(freqs_cos),
                         np.asarray(freqs_sin), mask, np.asarray(wq),
                         np.asarray(wk), np.asarray(wv), np.asarray(wo), uniq)
    res = run_bass_kernel_spmd(nc, in_maps, core_ids=list(range(NC)),
                               trace=trace)
    out = np.concatenate([res.results[c]["out"] for c in range(NC)], axis=0)
    return out.reshape(1, S, D).astype(np.float32), res


def kernel(x, freqs_cos, freqs_sin, mask, wq, wk, wv, wo):
    out, _ = run(x, freqs_cos, freqs_sin, mask, wq, wk, wv, wo, trace=False)
    return out


# revision 60
# speedup vs baseline: 1.0781x; 1.0781x over previous
"""Trainium2 Bass kernel for GQA attention (B=1, S=2048, D=2048, 32 Q heads,
8 KV heads, head_dim 64), 8-way tensor parallel over heads.

v2: fully software-pipelined single pass.
  - Core c owns Q heads 4c..4c+3 and KV head c (GQA maps exactly).
  - Emission interleaves projection (per 512-col n-group), attention
    (per q-group g=n, heads pair-interleaved), and the wo epilogue so the
    Scalar engine (softmax exp, the critical resource) runs dense from ~10us.
  - Scores S^T[k',q] = K^T Q are K=64 matmuls: two per PSUM pair run
    CONCURRENTLY in disjoint PE row-groups via tile_position (0,0)/(64,0),
    with K and Q duplicated on partitions 64..127.
  - exp is trimmed by the causal lead offset of the first tile of each pair
    (tiles sorted by lead desc); masked lead columns are zeroed by the
    multiplicative mask tiles (pt tiles memset once to avoid stale NaNs).
  - Softmax denominator rides as a 65th row of [V|1]^T P^T; normalization is
    reciprocal (DVE, direct from PSUM) -> bf16 -> ones[1,64]^T @ rec PE
    broadcast -> DVE multiply. GpSimd runs mask multiplies + PSUM->SBUF
    copies; its FIFO ends with the two AllToAll collectives only.
  - Two AllToAlls (heads 0-1, then 2-3); wo accumulates all 16 contraction
    tiles in PSUM across the A2A#2 wait (no SBUF accumulator).
  - xt is loaded as 64 [128,512] chunks (n-major) and its SBUF slots are
    reused for wo column chunks once the projection consumed them.
"""

import os
import sys

import numpy as np

for _p in ("/opt/trn_rl_repo", "/root/.axon_site/_ro/trn_rl_repo"):
    if os.path.isdir(_p) and _p not in sys.path:
        sys.path.insert(0, _p)

import ml_dtypes  # noqa: E402

from concourse import bacc, mybir, tile  # noqa: E402
from concourse.bass_utils import run_bass_kernel_spmd  # noqa: E402

BF16 = mybir.dt.bfloat16
F32 = mybir.dt.float32

S = 2048          # sequence length
D = 2048          # model dim
HD = 64           # head dim
NH = 32           # query heads
NKV = 8           # kv heads
NC = 8            # cores
HL = NH // NC     # q heads per core = 4
P = 128
QG = 512          # q-group width (score-tile free dim)
NG = S // QG      # 4 q groups
NT = S // P       # 16 k'-tiles
KD = D // P       # 16 contraction tiles for D-reductions
SR = S // NC      # 256 output rows per core
NE = D // QG      # 4 output column chunks

_bf = ml_dtypes.bfloat16


def _classify_mask(mask):
    """Per-tile slot plan. A slot computes scores for one k'-tile for TWO
    heads at once (partition halves, concurrent PE row groups). Per q-group
    g: non-skip tiles sorted by causal lead desc. Per tile: exp offset,
    duplicated [m_t|m_t] multiplicative mask index (None if fully passing)
    and multiply width."""
    mexp = np.exp(np.minimum(mask.astype(np.float64), 50.0)).astype(np.float32).T
    uniq = []
    uniq_keys = {}
    slots = {}
    for g in range(NG):
        sl = []
        for t in range(NT):
            tl = mexp[P * t:P * (t + 1), QG * g:QG * (g + 1)]
            if np.all(tl == 0.0):
                continue
            if np.all(tl == 1.0):
                sl.append((t, 0, None, 0))
                continue
            live = np.where((tl != 0.0).any(axis=0))[0]
            lead = (int(live[0]) // 8) * 8
            ne = np.where((tl != 1.0).any(axis=0))[0]
            w = min(QG, ((int(ne[-1]) + 1) + 3) // 4 * 4)
            comb = np.concatenate([tl, tl], axis=1).astype(_bf)
            key = comb.tobytes()
            if key not in uniq_keys:
                uniq_keys[key] = len(uniq)
                uniq.append(comb)
            sl.append((t, lead, uniq_keys[key], w))
        slots[g] = sorted(sl, key=lambda s: (-s[1], s[0]))
    return slots, uniq


def _build_nc(slots, n_uniq, dbg=False):
    nc = bacc.Bacc("TRN2", target_bir_lowering=False, debug=False,
                   num_devices=NC)

    xt_d = nc.dram_tensor("xt", [D, S], BF16, kind="ExternalInput")
    wqkv_d = nc.dram_tensor("wqkv", [D, HL * HD + 2 * HD], BF16,
                            kind="ExternalInput")
    wo_d = nc.dram_tensor("wo", [D, D], BF16, kind="ExternalInput")
    cos_d = nc.dram_tensor("cos2", [P, S], BF16, kind="ExternalInput")
    sin_d = nc.dram_tensor("sin2", [P, S], BF16, kind="ExternalInput")
    ident_d = nc.dram_tensor("ident", [P, P], BF16, kind="ExternalInput")
    pswap_d = nc.dram_tensor("pswap", [P, P], BF16, kind="ExternalInput")
    pkd_d = nc.dram_tensor("pkd", [P, P], BF16, kind="ExternalInput")
    pks_d = nc.dram_tensor("pks", [P, P], BF16, kind="ExternalInput")
    mt_d = None
    if n_uniq:
        mt_d = nc.dram_tensor("mtiles", [n_uniq, P, 2 * QG], BF16,
                              kind="ExternalInput")
    out_d = nc.dram_tensor("out", [SR, D], BF16, kind="ExternalOutput")
    dbg_d = {}
    if dbg:
        dbg_d["kt"] = nc.dram_tensor("dbg_kt", [NG, P, QG], BF16,
                                     kind="ExternalOutput")
        dbg_d["qt"] = nc.dram_tensor("dbg_qt", [2, NG, P, 2, QG], BF16,
                                     kind="ExternalOutput")
        dbg_d["v"] = nc.dram_tensor("dbg_v", [NT, P, HD + 1], BF16,
                                    kind="ExternalOutput")
        dbg_d["asb"] = nc.dram_tensor("dbg_asb", [HL, NG, HD, QG], BF16,
                                      kind="ExternalOutput")
        dbg_d["att"] = nc.dram_tensor("dbg_att", [HL, NG, HD + 1, QG], F32,
                                      kind="ExternalOutput")

    with tile.TileContext(nc) as tc:
        with (
            tc.tile_pool(name="xtp", bufs=1) as xtp,      # xt chunks then wo chunks
            tc.tile_pool(name="const", bufs=1) as const,
            tc.tile_pool(name="work", bufs=3) as work,
            tc.tile_pool(name="pt", bufs=4) as ptpool,
            tc.tile_pool(name="ps_sc", bufs=2, space="PSUM") as ps_sc,   # scores: [128,1024] f32 = 2 banks ea
            tc.tile_pool(name="ps_pj", bufs=2, space="PSUM") as ps_pj,   # proj/sw/pv/rec64: 1 bank ea
            tc.tile_pool(name="ps_att", bufs=2, space="PSUM") as ps_att,  # att accum: 1 bank ea
            tc.tile_pool(name="dram", bufs=1, space="DRAM") as dram,
        ):
            # ---- bulk loads. The Sync sequencer pays ~600ns per DMA trigger,
            # so triggers are few+fat; wq/wkv ride the Activation HWDGE queue
            # (idle until the first exp). xt n=0 is split thin so the first
            # projection group lands in ~8us; the rest is one fat DMA per k.
            wqkv_sb = [const.tile([P, HL * HD + 2 * HD], BF16, tag=f"wq{k}",
                                  name=f"wq{k}") for k in range(KD)]
            xt0 = [xtp.tile([P, 2 * QG], BF16, tag=f"x0{k}", name=f"xt0_{k}")
                   for k in range(KD)]
            for k in range(KD):
                nc.sync.dma_start(wqkv_sb[k][:],
                                  wqkv_d.ap()[P * k:P * (k + 1), :])
                nc.sync.dma_start(xt0[k][:],
                                  xt_d.ap()[P * k:P * (k + 1), 0:2 * QG])
            cos2 = const.tile([P, S], BF16)
            sin2 = const.tile([P, S], BF16)
            ident = const.tile([P, P], BF16)
            pswap = const.tile([P, P], BF16)
            pkd = const.tile([P, P], BF16)
            pks = const.tile([P, P], BF16)
            nc.sync.dma_start(cos2[:], cos_d.ap())
            nc.sync.dma_start(sin2[:], sin_d.ap())
            nc.sync.dma_start(ident[:], ident_d.ap())
            nc.sync.dma_start(pswap[:], pswap_d.ap())
            nc.sync.dma_start(pkd[:], pkd_d.ap())
            nc.sync.dma_start(pks[:], pks_d.ap())
            xtR = [xtp.tile([P, 2 * QG], BF16, tag=f"b{k}", name=f"xtR{k}")
                   for k in range(KD)]
            for k in range(KD):
                nc.sync.dma_start(xtR[k][:],
                                  xt_d.ap()[P * k:P * (k + 1), 2 * QG:S])
            mtt = [const.tile([P, 2, QG], BF16, tag=f"mt{u}", name=f"mt{u}")
                   for u in range(n_uniq)]
            for u in range(n_uniq):
                nc.sync.dma_start(mtt[u][:], mt_d.ap()[u])
            wo_sb = [None] * KD

            def xt_slice(k, n):
                if n < 2:
                    return xt0[k][:, QG * n:QG * (n + 1)]
                return xtR[k][:, QG * (n - 2):QG * (n - 1)]

            # ---- small consts / memsets (gpsimd is idle at start) ----
            ones = const.tile([1, HD], BF16)
            nc.gpsimd.memset(ones[:], 1.0)
            pts_tiles = [ptpool.tile([P, 2, QG], BF16, tag="pt",
                                     name=f"pt{i}") for i in range(4)]
            for t_ in pts_tiles:
                nc.gpsimd.memset(t_[:], 0.0)
            v_sb = [const.tile([P, HD + 1], BF16, tag=f"v{t}", name=f"v{t}")
                    for t in range(NT)]
            for t in range(NT):
                nc.gpsimd.memset(v_sb[t][:, HD:HD + 1], 1.0)

            # persistent projection outputs (fine-grained tiles for dep tracking)
            kt = [const.tile([P, QG], BF16, tag=f"kt{n}", name=f"kt{n}")
                  for n in range(NG)]
            qt = [[const.tile([P, QG], BF16, tag=f"qt{m}_{n}",
                              name=f"qt{m}_{n}") for n in range(NG)]
                  for m in range(2)]

            a2a_in = [dram.tile([NC, 2 * HD, SR], BF16, tag=f"a2ai{i}",
                                name=f"a2ai{i}") for i in range(2)]
            a2a_out = [dram.tile([NC, 2 * HD, SR], BF16, tag=f"a2ao{i}",
                                 name=f"a2ao{i}") for i in range(2)]

            pt_i = [0]

            def proj(m, n):
                """m=0: q heads 0-1; m=1: q heads 2-3; m=2: K|V.
                RoPE: rot = raw*cos + perm(raw)*sin, the permutation done on
                the PE. For K|V the permutations also duplicate K onto both
                partition halves, so kt needs no SBUF-shift DMA."""
                nsl = slice(QG * n, QG * (n + 1))
                ps = ps_pj.tile([P, QG], F32, tag="pj", name=f"pj{m}_{n}")
                for k in range(KD):
                    lhsT = (wqkv_sb[k][:, P * m:P * (m + 1)] if m < 2
                            else wqkv_sb[k][:, HL * HD:HL * HD + 2 * HD])
                    nc.tensor.matmul(ps[:, 0:QG], lhsT, xt_slice(k, n),
                                     start=(k == 0), stop=(k == KD - 1))
                raw = work.tile([P, QG], BF16, tag="raw")
                nc.vector.tensor_copy(raw[:], ps[:, 0:QG])
                if m < 2:
                    sw = ps_pj.tile([P, QG], F32, tag="pj", name=f"sw{m}_{n}")
                    nc.tensor.matmul(sw[:], pswap[:], raw[:], start=True,
                                     stop=True)
                    t1 = work.tile([P, QG], BF16, tag="t1")
                    nc.gpsimd.tensor_mul(t1[:], raw[:], cos2[:, nsl])
                    t2 = work.tile([P, QG], BF16, tag="t2")
                    nc.vector.tensor_mul(t2[:], sw[:], sin2[:, nsl])
                    nc.vector.tensor_add(qt[m][n][:], t1[:], t2[:])
                else:
                    kd = ps_pj.tile([P, QG], F32, tag="pj", name=f"kd{n}")
                    nc.tensor.matmul(kd[:], pkd[:], raw[:], start=True,
                                     stop=True)
                    ks = ps_pj.tile([P, QG], F32, tag="pj", name=f"ks{n}")
                    nc.tensor.matmul(ks[:], pks[:], raw[:], start=True,
                                     stop=True)
                    t1 = work.tile([P, QG], BF16, tag="t1")
                    nc.vector.tensor_mul(t1[:], kd[:], cos2[:, nsl])
                    t2 = work.tile([P, QG], BF16, tag="t2")
                    nc.vector.tensor_mul(t2[:], ks[:], sin2[:, nsl])
                    nc.vector.tensor_add(kt[n][:], t1[:], t2[:])
                    for j in range(4):
                        t = 4 * n + j
                        pv = ps_pj.tile([P, HD], BF16, tag="pj",
                                        name=f"pv{t}")
                        nc.tensor.transpose(pv[:],
                                            raw[HD:P, P * j:P * (j + 1)],
                                            ident[HD:P, 0:HD])
                        nc.vector.tensor_copy(v_sb[t][:, 0:HD], pv[:])

            def attn_slot(mq, g, si, attA, attB):
                t, off, u, w = slots[g][si]
                last = si == len(slots[g]) - 1
                tsl = slice(P * (t % 4), P * (t % 4) + P)
                psc = ps_sc.tile([P, 2, QG], F32, tag="sc",
                                 name=f"s{mq}_{g}_{si}")
                for i in range(2):
                    nc.tensor.matmul(
                        psc[:, i, :],
                        kt[t // 4][HD * i:HD * (i + 1), tsl],
                        qt[mq][g][HD * i:HD * (i + 1), :],
                        start=True, stop=True,
                        tile_position=(HD * i, 0))
                pts = pts_tiles[pt_i[0] % 4]
                pt_i[0] += 1
                nc.scalar.activation(pts[:, :, off:QG], psc[:, :, off:QG],
                                     mybir.ActivationFunctionType.Exp,
                                     scale=0.125)
                if u is not None:
                    nc.vector.tensor_mul(pts[:, :, 0:w], pts[:, :, 0:w],
                                         mtt[u][:, :, 0:w])
                nc.tensor.matmul(attA[:], v_sb[t][:], pts[:, 0, :],
                                 start=(si == 0), stop=last)
                nc.tensor.matmul(attB[:], v_sb[t][:], pts[:, 1, :],
                                 start=(si == 0), stop=last)

            def attn_fin(h, g, att):
                if dbg:
                    attf = work.tile([HD + 1, QG], F32, tag="attf")
                    nc.vector.tensor_copy(attf[:], att[:])
                    nc.sync.dma_start(dbg_d["att"].ap()[h, g], attf[:])
                # den row sits at att partition HD; the custom-DVE reciprocal
                # drops nonzero base partitions, so DMA it to partition 0.
                den = work.tile([1, QG], F32, tag="den")
                nc.scalar.copy(den[:], att[HD:HD + 1, :])
                rec = work.tile([1, QG], F32, tag="rec")
                nc.vector.reciprocal_approx_fast(rec[:], den[:])
                recb = work.tile([1, QG], BF16, tag="recb")
                nc.vector.tensor_copy(recb[:], rec[:])
                rec64 = ps_pj.tile([HD, QG], F32, tag="pj",
                                   name=f"rb{h}_{g}")
                nc.tensor.matmul(rec64[:], ones[:], recb[:],
                                 start=True, stop=True)
                rec64s = work.tile([HD, QG], BF16, tag="rec64s")
                nc.vector.tensor_copy(rec64s[:], rec64[:])
                asb = work.tile([HD, QG], BF16, tag="asb")
                nc.vector.tensor_mul(asb[:], att[0:HD, :], rec64s[:])
                if dbg:
                    nc.sync.dma_start(dbg_d["asb"].ap()[h, g], asb[:])
                buf = a2a_in[h // 2]
                hr = HD * (h % 2)
                nc.sync.dma_start(buf[2 * g, hr:hr + HD, :], asb[:, 0:SR])
                nc.sync.dma_start(buf[2 * g + 1, hr:hr + HD, :],
                                  asb[:, SR:2 * SR])

            def attn_hpair(ha, hb, g, mask_eng):
                """Head-pair chains for (ha, hb) = (2mq, 2mq+1) on group g;
                each slot computes both heads concurrently. Returns a
                finisher closure to emit later (FIFO slack)."""
                mq = ha // 2
                attA = ps_att.tile([HD + 1, QG], F32, tag="att",
                                   name=f"att{ha}_{g}")
                attB = ps_att.tile([HD + 1, QG], F32, tag="att",
                                   name=f"att{hb}_{g}")
                for si in range(len(slots[g])):
                    attn_slot(mq, g, si, attA, attB)

                def fin():
                    attn_fin(ha, g, attA)
                    attn_fin(hb, g, attB)
                return fin



            # ================= main pipeline =================
            # Phase 1: heads 0-1 across all groups -> A2A#1 fires early.
            # Phase 2: q23 projection + heads 2-3; wo epilogue overlaps A2A#2.
            # Finisher ordering rule: a chain's finisher (whose PE broadcast
            # frees its att PSUM slot) must be emitted BEFORE any later chain
            # that waits on that slot, else the PE FIFO deadlocks.
            fin_prev = None
            for n in range(NG):
                proj(2, n)                       # K|V for group n
                proj(0, n)                       # q heads 0-1
                if fin_prev is not None:
                    fin_prev()                   # (h0,h1, n-1)
                fin_prev = attn_hpair(0, 1, n, nc.gpsimd)
            proj(1, 0)                           # prefetch q23 group 0
            fin_prev()                           # (h0,h1, g=3)
            # wo even k-tiles (deferred so they don't steal xtR bandwidth)
            for i in range(NC):
                t_ = const.tile([P, D], BF16, tag=f"we{i}", name=f"wo{2 * i}")
                wo_sb[2 * i] = t_
                nc.sync.dma_start(t_[:], wo_d.ap()[P * 2 * i:P * (2 * i + 1), :])
            nc.gpsimd.collective_compute(
                "AllToAll", mybir.AluOpType.bypass,
                replica_groups=[list(range(NC))],
                ins=[a2a_in[0].opt()], outs=[a2a_out[0].opt()])
            # ao tiles reuse the (dead after proj n=0) xt0 slots
            ao_sb = [[xtp.tile([P, SR], BF16, tag=f"x0{2 * i + hp}",
                               name=f"ao{hp}_{i}") for i in range(NC)]
                     for hp in range(2)]
            fin_prev = None
            for n in range(NG):
                if n < NG - 1:
                    proj(1, n + 1)               # q heads 2-3, next group
                if n == NG - 1:
                    # all xtR readers emitted: wo odd k-tiles into the slots,
                    # then the ao0 loads (gated on A2A#1 completion)
                    for i in range(NC):
                        t_ = xtp.tile([P, D], BF16, tag=f"b{2 * i + 1}",
                                      name=f"wo{2 * i + 1}")
                        wo_sb[2 * i + 1] = t_
                        nc.sync.dma_start(
                            t_[:], wo_d.ap()[P * (2 * i + 1):
                                             P * (2 * i + 2), :])
                if fin_prev is not None:
                    fin_prev()                   # (h2,h3, n-1)
                fin_prev = attn_hpair(2, 3, n, nc.vector)
            fin_prev()                           # (h2,h3, g=3)
            for i in range(NC):
                nc.sync.dma_start(ao_sb[0][i][:], a2a_out[0][i])
            if dbg:
                for n in range(NG):
                    nc.sync.dma_start(dbg_d["kt"].ap()[n], kt[n][:])
                    for m in range(2):
                        nc.sync.dma_start(dbg_d["qt"].ap()[m, n], qt[m][n][:])
                for t in range(NT):
                    nc.sync.dma_start(dbg_d["v"].ap()[t], v_sb[t][:])

            # second collective
            nc.gpsimd.collective_compute(
                "AllToAll", mybir.AluOpType.bypass,
                replica_groups=[list(range(NC))],
                ins=[a2a_in[1].opt()], outs=[a2a_out[1].opt()])
            for i in range(NC):
                nc.sync.dma_start(ao_sb[1][i][:], a2a_out[1][i])

            # ---- wo epilogue: 8 PSUM accumulators over all 16 k-tiles ----
            po = []
            for j in range(2):
                t_ = ps_sc.tile([P, 2 * QG], F32, tag="sc", name=f"po_sc{j}")
                po += [t_[:, 0:QG], t_[:, QG:2 * QG]]
            for j in range(2):
                t_ = ps_att.tile([P, QG], F32, tag="att", name=f"po_at{j}")
                po.append(t_[:])
            for j in range(2):
                t_ = ps_pj.tile([P, QG], F32, tag="pj", name=f"po_pj{j}")
                po.append(t_[:])
            # po index = sm*NE + ec. Half 0 overlaps the A2A#2 wait; half 1
            # finishes each accumulator in turn so the stores pipeline out.
            for i in range(NC):
                for sm in range(SR // P):
                    for ec in range(NE):
                        nc.tensor.matmul(
                            po[sm * NE + ec],
                            ao_sb[0][i][:, P * sm:P * (sm + 1)],
                            wo_sb[2 * i][:, QG * ec:QG * (ec + 1)],
                            start=(i == 0), stop=False,
                            skip_group_check=True)
            for sm in range(SR // P):
                for ec in range(NE):
                    for i in range(NC):
                        nc.tensor.matmul(
                            po[sm * NE + ec],
                            ao_sb[1][i][:, P * sm:P * (sm + 1)],
                            wo_sb[2 * i + 1][:, QG * ec:QG * (ec + 1)],
                            start=False, stop=(i == NC - 1),
                            skip_group_check=True)
                    osb = work.tile([P, QG], BF16, tag="osb")
                    nc.vector.tensor_copy(osb[:], po[sm * NE + ec])
                    nc.sync.dma_start(
                        out_d.ap()[P * sm:P * (sm + 1),
                                   QG * ec:QG * (ec + 1)], osb[:])

    nc.compile()
    return nc


_CACHE = {}


def _get_compiled(mask):
    slots, uniq = _classify_mask(mask)
    key = tuple(sorted((g, tuple(sl)) for g, sl in slots.items()))
    if key not in _CACHE:
        _CACHE[key] = _build_nc(slots, len(uniq))
    return _CACHE[key], uniq


def _host_prep(x, freqs_cos, freqs_sin, mask, wq, wk, wv, wo, uniq):
    xt = np.ascontiguousarray(x[0].T).astype(_bf)
    perm = np.concatenate([np.arange(0, HD, 2), np.arange(1, HD, 2)])
    cosT = np.ascontiguousarray(freqs_cos.T)            # [32, S]
    sinT = np.ascontiguousarray(freqs_sin.T)
    cos2 = np.tile(cosT, (4, 1)).astype(_bf)            # [128, S]
    sin2 = np.tile(np.concatenate([-sinT, sinT], axis=0), (2, 1)).astype(_bf)
    ident = np.zeros((P, P), dtype=_bf)
    ident[0:HD, 0:HD] = np.eye(HD, dtype=_bf)
    ident[HD:P, 0:HD] = np.eye(HD, dtype=_bf)
    pswap = np.zeros((P, P), dtype=_bf)
    pkd = np.zeros((P, P), dtype=_bf)
    pks = np.zeros((P, P), dtype=_bf)
    for i in range(P):
        b, r = i // 32, i % 32
        pswap[32 * (b ^ 1) + r, i] = 1
        h64 = i % HD
        pkd[h64, i] = 1
        b2, r2 = h64 // 32, h64 % 32
        pks[32 * (b2 ^ 1) + r2, i] = 1
    wo_b = np.ascontiguousarray(wo).astype(_bf)
    mt = (np.stack(uniq, axis=0) if uniq
          else np.zeros((0, P, 2 * QG), dtype=_bf))

    in_maps = []
    for c in range(NC):
        qcols = np.concatenate(
            [HD * (HL * c + h) + perm for h in range(HL)])
        wqkv_c = np.ascontiguousarray(np.concatenate(
            [wq[:, qcols], wk[:, HD * c + perm],
             wv[:, HD * c:HD * (c + 1)]], axis=1)).astype(_bf)
        m = {"xt": xt, "wqkv": wqkv_c,
             "wo": wo_b, "cos2": cos2, "sin2": sin2, "ident": ident,
             "pswap": pswap, "pkd": pkd, "pks": pks}
        if len(uniq):
            m["mtiles"] = mt
        in_maps.append(m)
    return in_maps


def run(x, freqs_cos, freqs_sin, mask, wq, wk, wv, wo, trace=False):
    x = np.asarray(x, dtype=np.float32)
    mask = np.asarray(mask, dtype=np.float32)
    nc, uniq = _get_compiled(mask)
    in_maps = _host_prep(np.asarray(x), np.asarray(freqs_cos),
                         np.asarray(freqs_sin), mask, np.asarray(wq),
                         np.asarray(wk), np.asarray(wv), np.asarray(wo), uniq)
    res = run_bass_kernel_spmd(nc, in_maps, core_ids=list(range(NC)),
                               trace=trace)
    out = np.concatenate([np.asarray(res.results[c]["out"], dtype=np.float32)
                          for c in range(NC)], axis=0)
    return out.reshape(1, S, D), res


def kernel(x, freqs_cos, freqs_sin, mask, wq, wk, wv, wo):
    out, _ = run(x, freqs_cos, freqs_sin, mask, wq, wk, wv, wo, trace=False)
    return out
